# revision 26
# baseline (speedup 1.0000x reference)
"""Trainium2 Bass kernel for nn_Block_34256659153605 (dual-branch linear-attention
transformer block). Data-parallel over batch B=8 across 8 NeuronCores; each core
runs the full block for one batch item.

v1 (this file) vs baseline:
  - q-softmax reciprocal moved DVE -> ScalarE activation (was 215us of DVE).
  - LN eviction: single bf16 copy (no f32 h + gpsimd cast); apply is two bf16
    DVE tensor_tensor ops against bf16 broadcast rows.
  - k-softmax denominator folded into the ctx matmul via a ones-column in vt
    (kills the per-tile sk matmuls + phase-2 transposes).
  - pos embeddings for the self-attention branches (and the x-branch cross-attn
    query) folded on host into the inputs (kills 80 identity matmuls).
  - DMA batching: chunk loads/stores and broadcast reads are single 3D-AP DMAs.
  - MLP fc2 runs ht-outer with 4 concurrent PSUM accumulation groups so u tiles
    free early.
"""

import os
import sys
import numpy as np

if "/opt/trn_rl_repo" not in sys.path:
    sys.path.insert(0, "/opt/trn_rl_repo")

import ml_dtypes
from contextlib import ExitStack

import concourse.bass as bass
import concourse.mybir as mybir
import concourse.tile as tile
from concourse import bacc, library_config
from concourse.masks import make_identity

P = 128
C = 512
H = 4
HID = 4 * C
CT = C // P          # 4 channel blocks
HT = HID // P        # 16 hidden blocks
FD = 512             # token chunk size
EPS = 1e-5

bf16 = mybir.dt.bfloat16
f32 = mybir.dt.float32
AF = mybir.ActivationFunctionType
ALU = mybir.AluOpType

ATTN_W = ["sa_q", "sa_k", "sa_v", "sa_r", "ca_q", "ca_k", "ca_v", "ca_r"]


def build_nc(N=2048, ln_affine=False, biases=frozenset(), dbg=False):
    NCH = N // FD
    nc = bacc.Bacc("TRN2", debug=False)

    dr = {}
    def din(name, shape, dt, kind="ExternalInput"):
        if dbg and kind == "Internal":
            kind = "ExternalOutput"
        dr[name] = nc.dram_tensor(name, shape, dt, kind=kind).ap()

    din("xT_bf", (C, N), bf16)   # raw x^T (v-proj input + resid)
    din("yT_bf", (C, N), bf16)
    din("xP_bf", (C, N), bf16)   # (x+pos_x)^T (q/k-proj input for xsa)
    din("yP_bf", (C, N), bf16)
    din("qP_bf", (C, N), bf16)   # (q+pos_x)^T (q-proj input for xca)
    for w in ATTN_W:
        din(w + "_w", (C, C), bf16)
    din("mlp_w1", (C, HID), bf16)
    din("mlp_w2", (HID, C), bf16)
    din("pq_ca_y", (C, N), bf16)             # (pos_y @ ca_q_w)^T
    din("pk_ca_x", (N, C), bf16)             # pos_x @ ca_k_w
    din("pk_ca_y", (N, C), bf16)
    for bn in biases:
        din("b_" + bn, (1, HID if bn == "mlp1" else C), bf16)
    if ln_affine:
        din("ln_g", (C,), f32)
        din("ln_b", (C,), f32)
    for nm in ["z_osa", "z_oca", "z_oo", "z_ysa", "z_yca"]:
        din(nm, (C, N), bf16, kind="Internal")
    out_d = nc.dram_tensor("yOT", (C, N), f32, kind="ExternalOutput").ap()

    def bcast_rows(a):
        """DRAM AP slice (r, F) -> broadcast AP (128, r, F)."""
        return bass.AP(tensor=a.tensor, offset=a.offset,
                       ap=[[0, P]] + [list(d) for d in a.ap])

    with tile.TileContext(nc) as tc, ExitStack() as ctx:
        consts = ctx.enter_context(tc.tile_pool(name="consts", bufs=1))
        a16 = ctx.enter_context(tc.tile_pool(name="a16", bufs=2))
        a32 = ctx.enter_context(tc.tile_pool(name="a32", bufs=2))
        pmm = ctx.enter_context(tc.tile_pool(name="pmm", bufs=4, space="PSUM"))
        pcx = ctx.enter_context(tc.tile_pool(name="pcx", bufs=2, space="PSUM"))
        psm = ctx.enter_context(tc.tile_pool(name="psm", bufs=2, space="PSUM"))

        nc.gpsimd.load_library(library_config.attn)

        # ---------------- persistent constants ----------------
        def wload(name, dram, nblk, fd):
            t = consts.tile([P, nblk, fd], bf16, name=name)
            nc.sync.dma_start(out=t, in_=dram.rearrange("(i p) c -> p i c", p=P))
            return t

        wsb = {w: wload("w_" + w, dr[w + "_w"], CT, C) for w in ATTN_W}
        w1sb = wload("w_mlp1", dr["mlp_w1"], CT, HID)
        w2sb = wload("w_mlp2", dr["mlp_w2"], HT, C)

        id_bf = consts.tile([P, P], bf16, name="id_bf")
        make_identity(nc, id_bf)
        ones_bf = consts.tile([P, 1], bf16, name="ones_bf")
        nc.vector.memset(ones_bf, 1.0)
        ones32 = consts.tile([P, 32], bf16, name="ones32")
        nc.vector.memset(ones32, 1.0)
        ones_row = consts.tile([1, FD], bf16, name="ones_row")
        nc.vector.memset(ones_row, 1.0)
        eps_t = consts.tile([P, 1], f32, name="eps_t")
        nc.vector.memset(eps_t, EPS)
        brow = {}
        for bn in biases:
            bt = consts.tile([1, HID if bn == "mlp1" else C], bf16, name="br_" + bn)
            nc.sync.dma_start(out=bt, in_=dr["b_" + bn])
            brow[bn] = bt
        if ln_affine:
            g_col = consts.tile([P, CT], f32, name="g_col")
            b_col = consts.tile([P, CT], f32, name="b_col")
            nc.sync.dma_start(out=g_col, in_=dr["ln_g"].rearrange("(i p) -> p i", p=P))
            nc.sync.dma_start(out=b_col, in_=dr["ln_b"].rearrange("(i p) -> p i", p=P))

        ct_view = lambda d: d.rearrange("(i p) n -> p i n", p=P)

        def load_ct_chunk(d, c, name):
            """One DMA: [P, CT, FD] tile = 4 channel-blocks of token chunk c."""
            tl = a16.tile([P, CT, FD], bf16, name=name, tag="ld16", bufs=4)
            nc.sync.dma_start(out=tl, in_=ct_view(d)[:, :, c * FD:(c + 1) * FD])
            return tl

        def bias_ct(ps, bn, blk):
            nc.tensor.matmul(ps, lhsT=brow[bn][0:1, blk * P:(blk + 1) * P],
                             rhs=ones_row, start=False, stop=True)

        def bias_nt(ps, bn):
            nc.tensor.matmul(ps, lhsT=ones_row[0:1, 0:P], rhs=brow[bn],
                             start=False, stop=True)

        # ---------------- layernorm (over channels) ----------------
        class LNState:
            def __init__(self, tag, zout_dr, final_f32):
                self.tag = tag
                self.zout = zout_dr
                self.final = final_f32

        def ln_chunk(st, hb_c, hsq_c, c):
            """Per-chunk LN: PE stats rows (partition 0) -> on-chip row math
            (rstd = exp(-0.5*ln(var+eps))) -> gpsimd partition_broadcast ->
            apply -> one z DMA. No DRAM round trips."""
            tag = st.tag
            s_ps = psm.tile([P, FD], f32, name=tag + "_sps", tag="sm")
            q_ps = psm.tile([P, FD], f32, name=tag + "_qps", tag="sm")
            for i in range(CT):
                nc.tensor.matmul(s_ps[0:1, :], lhsT=ones_bf, rhs=hb_c[i],
                                 start=(i == 0), stop=(i == CT - 1),
                                 tile_position=(0, 0))
            for i in range(CT):
                nc.tensor.matmul(q_ps[0:1, :], lhsT=ones_bf, rhs=hsq_c[i],
                                 start=(i == 0), stop=(i == CT - 1),
                                 tile_position=(0, 0))
            rowt = a32.tile([P, 3 * FD], f32, name=tag + "_rowt", tag="rowt", bufs=2)
            m_row = rowt[0:1, 0:FD]
            q_row = rowt[0:1, FD:2 * FD]
            t_row = rowt[0:1, 2 * FD:3 * FD]
            nc.scalar.activation(out=m_row, in_=s_ps[0:1, :], func=AF.Copy,
                                 scale=1.0 / C)
            nc.scalar.activation(out=q_row, in_=q_ps[0:1, :], func=AF.Copy,
                                 scale=1.0 / C)
            nc.vector.tensor_mul(out=t_row, in0=m_row, in1=m_row)
            nc.vector.tensor_sub(out=t_row, in0=q_row, in1=t_row)   # var
            nc.scalar.activation(out=t_row, in_=t_row, func=AF.Ln,
                                 bias=eps_t[0:1, 0:1], scale=1.0)
            rwb = a16.tile([P, 2 * FD], bf16, name=tag + "_rwb", tag="rwb", bufs=2)
            nc.scalar.activation(out=rwb[0:1, 0:FD], in_=t_row, func=AF.Exp,
                                 scale=-0.5)                        # rstd
            nc.vector.tensor_mul(out=rwb[0:1, FD:2 * FD], in0=m_row,
                                 in1=rwb[0:1, 0:FD])                # m*rstd
            rbm = a16.tile([P, 2 * FD], bf16, name=tag + "_rbm", tag="bcr", bufs=3)
            nc.gpsimd.partition_broadcast(rbm, rwb[0:1, :])
            zv = ct_view(st.zout)
            if st.final:
                zc = a32.tile([P, CT, FD], f32, name=tag + "_zf",
                              tag="zf32", bufs=2)
            else:
                zc = a16.tile([P, CT, FD], bf16, name=tag + "_z",
                              tag="z16", bufs=2)
            for i in range(CT):
                t1 = a16.tile([P, FD], bf16, name=tag + "_t1", tag="t1", bufs=4)
                nc.vector.tensor_mul(out=t1, in0=hb_c[i], in1=rbm[:, 0:FD])
                nc.vector.tensor_sub(out=zc[:, i, :], in0=t1,
                                     in1=rbm[:, FD:2 * FD])
                if ln_affine:
                    nc.vector.tensor_scalar(out=zc[:, i, :], in0=zc[:, i, :],
                                            scalar1=g_col[:, i:i + 1],
                                            scalar2=b_col[:, i:i + 1],
                                            op0=ALU.mult, op1=ALU.add)
            nc.sync.dma_start(out=zv[:, :, c * FD:(c + 1) * FD], in_=zc)

        def evict_h(tag, ps):
            hb = a16.tile([P, FD], bf16, name=tag + "_hb", tag="hb16", bufs=8)
            nc.vector.tensor_copy(out=hb, in_=ps)
            sq = a16.tile([P, FD], bf16, name=tag + "_hsq", tag="sq16", bufs=6)
            nc.vector.tensor_mul(out=sq, in0=hb, in1=hb)
            return hb, sq

        def resid_mm(ps, rt, stop):
            nc.tensor.matmul(ps, lhsT=id_bf, rhs=rt, start=False, stop=stop)

        # ---------------- efficient attention ----------------
        def eattn(tag, qin, kin, vin, W, posq, posk, resid, zout,
                  final_f32=False, bq=None, bk=None, bv=None, br=None):
            wq, wk, wv, wr = (wsb[W + "_q"], wsb[W + "_k"],
                              wsb[W + "_v"], wsb[W + "_r"])
            # ---- phase 1: kp / vp / ctx (+s_k ones column), token-tiled ----
            # ctx for heads {2h, 2h+1} shares one PSUM bank; a single
            # accumulation group per bank (start only on the very first MM).
            ctx_ps = [pcx.tile([P, 2, 132], f32, name=tag + "_ctx%d" % b, tag="cx")
                      for b in range(2)]
            pkv = dr[posk].rearrange("(cc tt p) c -> cc p tt c", p=P, tt=4) \
                if posk else None
            for c in range(NCH):
                kint = load_ct_chunk(dr[kin], c, tag + "_kin")
                vint = kint if vin == kin else load_ct_chunk(dr[vin], c, tag + "_vin")
                pkt = None
                if posk:
                    pkt = a16.tile([P, 4, C], bf16, name=tag + "_pk", tag="pk16", bufs=2)
                    nc.sync.dma_start(out=pkt, in_=pkv[c])
                for tt in range(4):
                    t = 4 * c + tt
                    kp = pmm.tile([P, FD], f32, name=tag + "_kp", tag="mm")
                    for i in range(CT):
                        nc.tensor.matmul(kp, lhsT=kint[:, i, tt * P:(tt + 1) * P],
                                         rhs=wk[:, i, :], start=(i == 0),
                                         stop=(i == CT - 1 and posk is None
                                               and bk is None))
                    if posk:
                        nc.tensor.matmul(kp, lhsT=id_bf, rhs=pkt[:, tt, :],
                                         start=False, stop=(bk is None))
                    if bk is not None:
                        bias_nt(kp, bk)
                    ek = a16.tile([P, FD], bf16, name=tag + "_ek", tag="kv16", bufs=5)
                    nc.scalar.activation(out=ek, in_=kp, func=AF.Exp)
                    vp = pmm.tile([P, FD], f32, name=tag + "_vp", tag="mm")
                    for i in range(CT):
                        nc.tensor.matmul(vp, lhsT=vint[:, i, tt * P:(tt + 1) * P],
                                         rhs=wv[:, i, :], start=(i == 0),
                                         stop=(i == CT - 1 and bv is None))
                    if bv is not None:
                        bias_nt(vp, bv)
                    vt = a16.tile([P, H, 132], bf16, name=tag + "_vt", tag="vt16", bufs=5)
                    nc.vector.tensor_copy(
                        out=vt[:, :, 0:128],
                        in_=vp.rearrange("p (h v) -> p h v", h=H))
                    nc.vector.memset(vt[:, :, 128:129], 1.0)
                    for h in range(H):
                        nc.tensor.matmul(ctx_ps[h // 2][:, h % 2, 0:129],
                                         lhsT=ek[:, h * P:(h + 1) * P],
                                         rhs=vt[:, h, 0:129],
                                         start=(t == 0 and h % 2 == 0),
                                         stop=(t == 4 * NCH - 1 and h % 2 == 1))
            # ---- phase 2: normalize ctx rows by the ones-column sum ----
            ctx_bf = []
            for h in range(H):
                rk = a32.tile([P, 1], f32, name=tag + "_rk", tag="rk", bufs=4)
                nc.vector.reciprocal(out=rk, in_=ctx_ps[h // 2][:, h % 2, 128:129])
                cb = a16.tile([P, P], bf16, name=tag + "_cbf", tag="cbf", bufs=8)
                nc.vector.tensor_scalar_mul(out=cb, in0=ctx_ps[h // 2][:, h % 2, 0:128],
                                            scalar1=rk[:, 0:1])
                ctx_bf.append(cb)
            # ---- phase 3: qp / q-softmax / att / reproj+resid, chunk ordered ----
            pqv = ct_view(dr[posq]) if posq else None
            rv = dr[resid]
            lst = LNState(tag, zout, final_f32)
            for c in range(NCH):
                qint = load_ct_chunk(dr[qin], c, tag + "_qin")
                pqt = None
                if posq:
                    pqt = a16.tile([P, CT, FD], bf16, name=tag + "_pq", tag="pk16", bufs=2)
                    nc.sync.dma_start(out=pqt, in_=pqv[:, :, c * FD:(c + 1) * FD])
                sq_ps = pcx.tile([P, FD], f32, name=tag + "_sq", tag="cx")
                eq = []
                for m in range(CT):
                    ps = pmm.tile([P, FD], f32, name=tag + "_qp", tag="mm")
                    for i in range(CT):
                        nc.tensor.matmul(ps, lhsT=wq[:, i, m * P:(m + 1) * P],
                                         rhs=qint[:, i, :], start=(i == 0),
                                         stop=(i == CT - 1 and posq is None
                                               and bq is None))
                    if posq:
                        nc.tensor.matmul(ps, lhsT=id_bf, rhs=pqt[:, m, :],
                                         start=False, stop=(bq is None))
                    if bq is not None:
                        bias_ct(ps, bq, m)
                    e = a16.tile([P, FD], bf16, name=tag + "_eq", tag="eq16", bufs=6)
                    nc.scalar.activation(out=e, in_=ps, func=AF.Exp)
                    eq.append(e)
                    nc.tensor.matmul(sq_ps[32 * m:32 * m + 32, :], lhsT=ones32,
                                     rhs=e, start=True, stop=True,
                                     tile_position=(0, 32 * m))
                # 1/sq: block-transpose the slab, one packed exact DVE
                # reciprocal (16 elems/partition), then rearranging SBUF->SBUF
                # DMA hops to partition-0 rows for partition_broadcast.
                sqT = a32.tile([P, FD], f32, name=tag + "_sqT", tag="rqs", bufs=2)
                nc.vector.transpose(out=sqT, in_=sq_ps)
                rq16 = a32.tile([P, 16], f32, name=tag + "_rq16", tag="rq16", bufs=2)
                nc.vector.reciprocal(out=rq16, in_=sqT[:, 0:FD:32])
                rq16b = a16.tile([P, 16], bf16, name=tag + "_rq16b", tag="rq16c", bufs=2)
                nc.vector.tensor_copy(out=rq16b, in_=rq16)
                rqb = []
                for h in range(H):
                    # flatten head h's packed [32,16] block to a partition-0
                    # row (packed order: offset = 16*p' + j for token 32j+p');
                    # the ab multiply unscrambles with a strided AP.
                    r0 = a16.tile([1, FD], bf16, name=tag + "_rq0_%d" % h,
                                  tag="rq0", bufs=6)
                    nc.scalar.dma_start(out=r0, in_=rq16b[32 * h:32 * h + 32, :])
                    b_ = a16.tile([P, FD], bf16, name=tag + "_rqb%d" % h,
                                  tag="bcr2", bufs=5)
                    nc.gpsimd.partition_broadcast(b_, r0)
                    rqb.append(b_)
                att = []
                for h in range(H):
                    aps = pmm.tile([P, FD], f32, name=tag + "_aps", tag="mm")
                    nc.tensor.matmul(aps, lhsT=ctx_bf[h], rhs=eq[h],
                                     start=True, stop=True)
                    ab = a16.tile([P, FD], bf16, name=tag + "_ab", tag="att16", bufs=6)
                    nc.vector.tensor_mul(
                        out=ab.rearrange("p (j q) -> p j q", j=16),
                        in0=aps.rearrange("p (j q) -> p j q", j=16),
                        in1=rqb[h].rearrange("p (q j) -> p j q", q=32))
                    att.append(ab)
                rts = load_ct_chunk(rv, c, tag + "_rt")
                hbc, hsqc = [], []
                for i in range(CT):
                    ps = pmm.tile([P, FD], f32, name=tag + "_rp", tag="mm")
                    for hh in range(CT):
                        nc.tensor.matmul(ps, lhsT=wr[:, hh, i * P:(i + 1) * P],
                                         rhs=att[hh], start=(hh == 0), stop=False)
                    resid_mm(ps, rts[:, i, :], stop=(br is None))
                    if br is not None:
                        bias_ct(ps, br, i)
                    hb_, sq_ = evict_h(tag, ps)
                    hbc.append(hb_)
                    hsqc.append(sq_)
                ln_chunk(lst, hbc, hsqc, c)

        # ---------------- MLP ----------------
        def mlp(tag, zin, zout, final_f32, b1=None, b2=None):
            lst = LNState(tag, zout, final_f32)
            for c in range(NCH):
                zint = load_ct_chunk(dr[zin], c, tag + "_zin")
                rts = zint  # resid source == fc1 input: reuse the same tile
                f2 = [pmm.tile([P, FD], f32, name=tag + "_f2_%d" % i, tag="mm")
                      for i in range(CT)]
                for ht in range(HT):
                    ps = pcx.tile([P, FD], f32, name=tag + "_f1", tag="cx")
                    for i in range(CT):
                        nc.tensor.matmul(ps, lhsT=w1sb[:, i, ht * P:(ht + 1) * P],
                                         rhs=zint[:, i, :], start=(i == 0),
                                         stop=(i == CT - 1 and b1 is None))
                    if b1 is not None:
                        bias_ct(ps, b1, ht)
                    ut = a16.tile([P, FD], bf16, name=tag + "_u", tag="u16", bufs=6)
                    if ht % 2 == 0:
                        nc.scalar.activation(out=ut, in_=ps, func=AF.Relu)
                    else:
                        nc.vector.tensor_scalar_max(out=ut, in0=ps, scalar1=0.0)
                    for i in range(CT):
                        nc.tensor.matmul(f2[i], lhsT=w2sb[:, ht, i * P:(i + 1) * P],
                                         rhs=ut, start=(ht == 0), stop=False)
                hbc, hsqc = [], []
                for i in range(CT):
                    resid_mm(f2[i], rts[:, i, :], stop=(b2 is None))
                    if b2 is not None:
                        bias_ct(f2[i], b2, i)
                    hb_, sq_ = evict_h(tag, f2[i])
                    hbc.append(hb_)
                    hsqc.append(sq_)
                ln_chunk(lst, hbc, hsqc, c)

        bb = lambda n: (n if n in biases else None)

        eattn("xsa", "xP_bf", "xP_bf", "xT_bf", "sa", None, None,
              "xT_bf", dr["z_osa"],
              bq=bb("sa_q"), bk=bb("sa_k"), bv=bb("sa_v"), br=bb("sa_r"))
        eattn("ysa", "yP_bf", "yP_bf", "yT_bf", "sa", None, None,
              "yT_bf", dr["z_ysa"],
              bq=bb("sa_q"), bk=bb("sa_k"), bv=bb("sa_v"), br=bb("sa_r"))
        eattn("xca", "qP_bf", "z_osa", "z_osa", "ca", None, "pk_ca_x",
              "z_osa", dr["z_oca"],
              bq=bb("ca_q"), bk=bb("ca_k"), bv=bb("ca_v"), br=bb("ca_r"))
        mlp("xml", "z_oca", dr["z_oo"], False, b1=bb("mlp1"), b2=bb("mlp2"))
        eattn("yca", "z_oo", "z_ysa", "z_ysa", "ca", "pq_ca_y", "pk_ca_y",
              "z_ysa", dr["z_yca"],
              bq=bb("ca_q"), bk=bb("ca_k"), bv=bb("ca_v"), br=bb("ca_r"))
        mlp("yml", "z_yca", out_d, True, b1=bb("mlp1"), b2=bb("mlp2"))

    nc.compile()
    return nc


# ======================= host side =======================

_NC_CACHE = {}
LAST_RESULT = None


def _get_nc(N, ln_affine, biases):
    key = (N, ln_affine, tuple(sorted(biases)))
    if key not in _NC_CACHE:
        _NC_CACHE[key] = build_nc(N, ln_affine, frozenset(biases))
    return _NC_CACHE[key]


def _bf(a):
    return np.ascontiguousarray(a.astype(ml_dtypes.bfloat16))


def host_prep(inputs, N):
    """Common (core-independent) in_map entries."""
    ws = {w: np.asarray(inputs[w + "_w"], np.float32) for w in ATTN_W}
    posx = np.asarray(inputs["pos_x"], np.float32)[0]  # (N, C)
    posy = np.asarray(inputs["pos_y"], np.float32)[0]
    m = {}
    for w in ATTN_W:
        m[w + "_w"] = _bf(ws[w])
    m["mlp_w1"] = _bf(np.asarray(inputs["mlp_w1"], np.float32))
    m["mlp_w2"] = _bf(np.asarray(inputs["mlp_w2"], np.float32))
    m["pq_ca_y"] = _bf((posy @ ws["ca_q"]).T)
    m["pk_ca_x"] = _bf(posx @ ws["ca_k"])
    m["pk_ca_y"] = _bf(posy @ ws["ca_k"])
    bias_arr = {"sa_q": "sa_q_b", "sa_k": "sa_k_b", "sa_v": "sa_v_b",
                "sa_r": "sa_r_b", "ca_q": "ca_q_b", "ca_k": "ca_k_b",
                "ca_v": "ca_v_b", "ca_r": "ca_r_b",
                "mlp1": "mlp_b1", "mlp2": "mlp_b2"}
    biases = set()
    for bn, an in bias_arr.items():
        arr = np.asarray(inputs[an], np.float32)
        if np.any(arr != 0):
            biases.add(bn)
            m["b_" + bn] = _bf(arr.reshape(1, -1))
    g = np.asarray(inputs["ln_g"], np.float32)
    b = np.asarray(inputs["ln_b"], np.float32)
    ln_affine = bool(np.any(g != 1) or np.any(b != 0))
    if ln_affine:
        m["ln_g"] = np.ascontiguousarray(g)
        m["ln_b"] = np.ascontiguousarray(b)
    return m, biases, ln_affine


def core_inputs(inputs, b, posx, posy):
    x = np.asarray(inputs["x"], np.float32)[b]
    y = np.asarray(inputs["y"], np.float32)[b]
    q = np.asarray(inputs["q"], np.float32)[b]
    return {"xT_bf": _bf(x.T), "yT_bf": _bf(y.T),
            "xP_bf": _bf((x + posx).T), "yP_bf": _bf((y + posy).T),
            "qP_bf": _bf((q + posx).T)}


def kernel(**inputs):
    from concourse import bass_utils
    N = np.asarray(inputs["x"]).shape[1]
    B = np.asarray(inputs["x"]).shape[0]
    common, biases, ln_affine = host_prep(inputs, N)
    nc = _get_nc(N, ln_affine, biases)
    posx = np.asarray(inputs["pos_x"], np.float32)[0]
    posy = np.asarray(inputs["pos_y"], np.float32)[0]
    in_maps = []
    for b in range(B):
        m = dict(common)
        m.update(core_inputs(inputs, b, posx, posy))
        in_maps.append(m)
    res = bass_utils.run_bass_kernel_spmd(nc, in_maps, core_ids=list(range(B)))
    global LAST_RESULT
    LAST_RESULT = res
    out = np.stack([r["yOT"].T for r in res.results], axis=0)
    return np.ascontiguousarray(out.astype(np.float32))


# revision 30
# speedup vs baseline: 1.3677x; 1.3677x over previous
"""Trainium2 Bass kernel for nn_Block_34256659153605 (dual-branch linear-attention
transformer block). Data-parallel over batch B=8 across 8 NeuronCores; each core
runs the full block for one batch item.

v1 (this file) vs baseline:
  - q-softmax reciprocal moved DVE -> ScalarE activation (was 215us of DVE).
  - LN eviction: single bf16 copy (no f32 h + gpsimd cast); apply is two bf16
    DVE tensor_tensor ops against bf16 broadcast rows.
  - k-softmax denominator folded into the ctx matmul via a ones-column in vt
    (kills the per-tile sk matmuls + phase-2 transposes).
  - pos embeddings for the self-attention branches (and the x-branch cross-attn
    query) folded on host into the inputs (kills 80 identity matmuls).
  - DMA batching: chunk loads/stores and broadcast reads are single 3D-AP DMAs.
  - MLP fc2 runs ht-outer with 4 concurrent PSUM accumulation groups so u tiles
    free early.
"""

import os
import sys
import numpy as np

if "/opt/trn_rl_repo" not in sys.path:
    sys.path.insert(0, "/opt/trn_rl_repo")

import ml_dtypes
from contextlib import ExitStack

import concourse.bass as bass
import concourse.mybir as mybir
import concourse.tile as tile
from concourse import bacc, library_config
from concourse.masks import make_identity

P = 128
C = 512
H = 4
HID = 4 * C
CT = C // P          # 4 channel blocks
HT = HID // P        # 16 hidden blocks
FD = 512             # token chunk size
EPS = 1e-5

bf16 = mybir.dt.bfloat16
f32 = mybir.dt.float32
f8 = mybir.dt.float8e4
SW = 32.0      # fp8 weight scale
SA = 16.0      # fp8 att scale
DR = mybir.MatmulPerfMode.DoubleRow
AF = mybir.ActivationFunctionType
ALU = mybir.AluOpType

ATTN_W = ["sa_q", "sa_k", "sa_v", "sa_r", "ca_q", "ca_k", "ca_v", "ca_r"]


def build_nc(N=2048, ln_affine=False, biases=frozenset(), dbg=False):
    assert not biases, "bias folding not supported in fp8 path"
    NCH = N // FD
    nc = bacc.Bacc("TRN2", debug=False)

    dr = {}
    def din(name, shape, dt, kind="ExternalInput"):
        if dbg and kind == "Internal":
            kind = "ExternalOutput"
        dr[name] = nc.dram_tensor(name, shape, dt, kind=kind).ap()

    din("xT_bf", (C, N), bf16)   # raw x^T (resid)
    din("yT_bf", (C, N), bf16)
    din("xT_f8", (C, N), f8)     # raw x^T (v-proj input)
    din("yT_f8", (C, N), f8)
    din("xP_f8", (C, N), f8)     # (x+pos_x)^T (q/k-proj input for xsa)
    din("yP_f8", (C, N), f8)
    din("qP_f8", (C, N), f8)     # (q+pos_x)^T (q-proj input for xca)
    for w in ATTN_W:
        din(w + "_w8", (C, C), f8)          # scaled by SW
    din("mlp_w1_8", (C, HID), f8)           # scaled by SW
    din("mlp_w2_8", (HID, C), f8)
    din("mlp_w1", (C, HID), bf16)
    din("mlp_w2", (HID, C), bf16)
    din("ca_r_wb", (C, C), bf16)
    din("pq_ca_y", (C, N), bf16)             # (pos_y @ ca_q_w)^T
    din("pk_ca_x", (N, C), bf16)             # pos_x @ ca_k_w
    din("pk_ca_y", (N, C), bf16)
    for bn in biases:
        din("b_" + bn, (1, HID if bn == "mlp1" else C), bf16)
    if ln_affine:
        din("ln_g", (C,), f32)
        din("ln_b", (C,), f32)
    for nm in ["z_osa", "z_oca", "z_oo", "z_ysa", "z_yca"]:
        din(nm, (C, N), bf16, kind="Internal")
    out_d = nc.dram_tensor("yOT", (C, N), f32, kind="ExternalOutput").ap()

    def bcast_rows(a):
        """DRAM AP slice (r, F) -> broadcast AP (128, r, F)."""
        return bass.AP(tensor=a.tensor, offset=a.offset,
                       ap=[[0, P]] + [list(d) for d in a.ap])

    with tile.TileContext(nc) as tc, ExitStack() as ctx:
        consts = ctx.enter_context(tc.tile_pool(name="consts", bufs=1))
        a16 = ctx.enter_context(tc.tile_pool(name="a16", bufs=2))
        a32 = ctx.enter_context(tc.tile_pool(name="a32", bufs=2))
        pmm = ctx.enter_context(tc.tile_pool(name="pmm", bufs=4, space="PSUM"))
        pcx = ctx.enter_context(tc.tile_pool(name="pcx", bufs=2, space="PSUM"))
        psm = ctx.enter_context(tc.tile_pool(name="psm", bufs=2, space="PSUM"))

        nc.gpsimd.load_library(library_config.attn)

        # ---------------- persistent constants ----------------
        def wload(name, dram, nblk, fd):
            t = consts.tile([P, nblk, fd], bf16, name=name)
            nc.sync.dma_start(out=t, in_=dram.rearrange("(i p) c -> p i c", p=P))
            return t

        def wload8(name, dram, nblk, fd):
            t = consts.tile([P, nblk, fd], f8, name=name)
            nc.sync.dma_start(out=t, in_=dram.rearrange("(i p) c -> p i c", p=P))
            return t

        wsb = {w: wload8("w_" + w, dr[w + "_w8"], CT, C) for w in ATTN_W}
        w1sb = wload8("w_mlp1", dr["mlp_w1_8"], CT, HID)
        w2sb = wload8("w_mlp2", dr["mlp_w2_8"], HT, C)
        w1sb_b = wload("w_mlp1b", dr["mlp_w1"], CT, HID)
        w2sb_b = wload("w_mlp2b", dr["mlp_w2"], HT, C)
        wr_ca_b = wload("w_ca_rb", dr["ca_r_wb"], CT, C)

        id_bf = consts.tile([P, P], bf16, name="id_bf")
        make_identity(nc, id_bf)
        # scaled identities: resid/pos id-matmuls must match the fp8 psum scale
        id_sw = consts.tile([P, P], bf16, name="id_sw")
        nc.vector.tensor_scalar_mul(out=id_sw, in0=id_bf, scalar1=SW)
        id_swa = consts.tile([P, P], bf16, name="id_swa")
        nc.vector.tensor_scalar_mul(out=id_swa, in0=id_bf, scalar1=SW * SA)
        id_sa = consts.tile([P, P], bf16, name="id_sa")
        nc.vector.tensor_scalar_mul(out=id_sa, in0=id_bf, scalar1=SA)
        ones_bf = consts.tile([P, 1], bf16, name="ones_bf")
        nc.vector.memset(ones_bf, 1.0)
        ones32 = consts.tile([P, 32], bf16, name="ones32")
        nc.vector.memset(ones32, 1.0)
        ones_row = consts.tile([1, FD], bf16, name="ones_row")
        nc.vector.memset(ones_row, 1.0)
        eps_t = consts.tile([P, 1], f32, name="eps_t")
        nc.vector.memset(eps_t, EPS)
        lnsa_t = consts.tile([P, 1], f32, name="lnsa_t")
        nc.vector.memset(lnsa_t, float(np.log(SA)))
        brow = {}
        for bn in biases:
            bt = consts.tile([1, HID if bn == "mlp1" else C], bf16, name="br_" + bn)
            nc.sync.dma_start(out=bt, in_=dr["b_" + bn])
            brow[bn] = bt
        if ln_affine:
            g_col = consts.tile([P, CT], f32, name="g_col")
            b_col = consts.tile([P, CT], f32, name="b_col")
            nc.sync.dma_start(out=g_col, in_=dr["ln_g"].rearrange("(i p) -> p i", p=P))
            nc.sync.dma_start(out=b_col, in_=dr["ln_b"].rearrange("(i p) -> p i", p=P))

        ct_view = lambda d: d.rearrange("(i p) n -> p i n", p=P)

        def load_ct_chunk(d, c, name):
            """One DMA: [P, CT, FD] tile = 4 channel-blocks of token chunk c."""
            tl = a16.tile([P, CT, FD], bf16, name=name, tag="ld16", bufs=4)
            nc.sync.dma_start(out=tl, in_=ct_view(d)[:, :, c * FD:(c + 1) * FD])
            return tl

        def load_f8_chunk(d, c, name):
            """f8 [P, CT, FD] chunk: direct load, or bf16 load + DVE cast."""
            if d.dtype == f8:
                tl = a16.tile([P, CT, FD], f8, name=name, tag="ld8", bufs=4)
                nc.sync.dma_start(out=tl,
                                  in_=ct_view(d)[:, :, c * FD:(c + 1) * FD])
                return tl
            tb = load_ct_chunk(d, c, name + "_b")
            tl = a16.tile([P, CT, FD], f8, name=name, tag="ld8", bufs=4)
            nc.vector.tensor_copy(out=tl, in_=tb)
            return tl

        def bias_ct(ps, bn, blk):
            nc.tensor.matmul(ps, lhsT=brow[bn][0:1, blk * P:(blk + 1) * P],
                             rhs=ones_row, start=False, stop=True)

        def bias_nt(ps, bn):
            nc.tensor.matmul(ps, lhsT=ones_row[0:1, 0:P], rhs=brow[bn],
                             start=False, stop=True)

        # ---------------- layernorm (over channels) ----------------
        class LNState:
            def __init__(self, tag, zout_dr, final_f32):
                self.tag = tag
                self.zout = zout_dr
                self.final = final_f32

        def ln_chunk(st, hb_c, hsq_c, c):
            """Per-chunk LN: PE stats rows (partition 0) -> on-chip row math
            (rstd = exp(-0.5*ln(var+eps))) -> gpsimd partition_broadcast ->
            apply -> one z DMA. No DRAM round trips."""
            tag = st.tag
            s_ps = psm.tile([P, FD], f32, name=tag + "_sps", tag="sm")
            q_ps = psm.tile([P, FD], f32, name=tag + "_qps", tag="sm")
            for i in range(CT):
                nc.tensor.matmul(s_ps[0:1, :], lhsT=ones_bf, rhs=hb_c[i],
                                 start=(i == 0), stop=(i == CT - 1),
                                 tile_position=(0, 0))
            for i in range(CT):
                nc.tensor.matmul(q_ps[0:1, :], lhsT=ones_bf, rhs=hsq_c[i],
                                 start=(i == 0), stop=(i == CT - 1),
                                 tile_position=(0, 0))
            rowt = a32.tile([P, 3 * FD], f32, name=tag + "_rowt", tag="rowt", bufs=2)
            m_row = rowt[0:1, 0:FD]
            q_row = rowt[0:1, FD:2 * FD]
            t_row = rowt[0:1, 2 * FD:3 * FD]
            nc.scalar.activation(out=m_row, in_=s_ps[0:1, :], func=AF.Copy,
                                 scale=1.0 / C)
            nc.scalar.activation(out=q_row, in_=q_ps[0:1, :], func=AF.Copy,
                                 scale=1.0 / C)
            nc.vector.tensor_mul(out=t_row, in0=m_row, in1=m_row)
            nc.vector.tensor_sub(out=t_row, in0=q_row, in1=t_row)   # var
            nc.scalar.activation(out=t_row, in_=t_row, func=AF.Ln,
                                 bias=eps_t[0:1, 0:1], scale=1.0)
            rwb = a16.tile([P, 2 * FD], bf16, name=tag + "_rwb", tag="rwb", bufs=2)
            nc.scalar.activation(out=rwb[0:1, 0:FD], in_=t_row, func=AF.Exp,
                                 scale=-0.5)                        # rstd
            nc.vector.tensor_mul(out=rwb[0:1, FD:2 * FD], in0=m_row,
                                 in1=rwb[0:1, 0:FD])                # m*rstd
            rbm = a16.tile([P, 2 * FD], bf16, name=tag + "_rbm", tag="bcr", bufs=3)
            nc.gpsimd.partition_broadcast(rbm, rwb[0:1, :])
            zv = ct_view(st.zout)
            if st.final:
                zc = a32.tile([P, CT, FD], f32, name=tag + "_zf",
                              tag="zf32", bufs=2)
            else:
                zc = a16.tile([P, CT, FD], bf16, name=tag + "_z",
                              tag="z16", bufs=2)
            for i in range(CT):
                t1 = a16.tile([P, FD], bf16, name=tag + "_t1", tag="t1", bufs=4)
                nc.vector.tensor_mul(out=t1, in0=hb_c[i], in1=rbm[:, 0:FD])
                nc.vector.tensor_sub(out=zc[:, i, :], in0=t1,
                                     in1=rbm[:, FD:2 * FD])
                if ln_affine:
                    nc.vector.tensor_scalar(out=zc[:, i, :], in0=zc[:, i, :],
                                            scalar1=g_col[:, i:i + 1],
                                            scalar2=b_col[:, i:i + 1],
                                            op0=ALU.mult, op1=ALU.add)
            nc.sync.dma_start(out=zv[:, :, c * FD:(c + 1) * FD], in_=zc)

        def evict_h(tag, ps, sc):
            hb = a16.tile([P, FD], bf16, name=tag + "_hb", tag="hb16", bufs=8)
            if sc == 1.0:
                nc.vector.tensor_copy(out=hb, in_=ps)
            else:
                nc.vector.tensor_scalar_mul(out=hb, in0=ps, scalar1=sc)
            sq = a16.tile([P, FD], bf16, name=tag + "_hsq", tag="sq16", bufs=6)
            nc.scalar.activation(out=sq, in_=ps, func=AF.Square, scale=sc)
            return hb, sq

        def resid_mm(ps, rt, stop):
            nc.tensor.matmul(ps, lhsT=id_bf, rhs=rt, start=False, stop=stop)

        # ---------------- efficient attention ----------------
        def eattn(tag, qin, kin, vin, W, posq, posk, resid, zout,
                  final_f32=False, bq=None, bk=None, bv=None, br=None,
                  f8r=True, wr_b=None):
            wq, wk, wv, wr = (wsb[W + "_q"], wsb[W + "_k"],
                              wsb[W + "_v"], wsb[W + "_r"])
            # ---- phase 1: kp / vp / ctx (+s_k ones column), token-tiled ----
            # ctx for heads {2h, 2h+1} shares one PSUM bank; a single
            # accumulation group per bank (start only on the very first MM).
            ctx_ps = [pcx.tile([P, 2, 132], f32, name=tag + "_ctx%d" % b, tag="cx")
                      for b in range(2)]
            pkv = dr[posk].rearrange("(cc tt p) c -> cc p tt c", p=P, tt=4) \
                if posk else None
            for c in range(NCH):
                kint = load_f8_chunk(dr[kin], c, tag + "_kin")
                vint = kint if vin == kin else load_f8_chunk(dr[vin], c, tag + "_vin")
                pkt = None
                if posk:
                    pkt = a16.tile([P, 4, C], bf16, name=tag + "_pk", tag="pk16", bufs=2)
                    nc.sync.dma_start(out=pkt, in_=pkv[c])
                for tt in range(4):
                    t = 4 * c + tt
                    kp = pmm.tile([P, FD], f32, name=tag + "_kp", tag="mm")
                    for a in range(2):
                        nc.tensor.matmul(
                            kp, lhsT=kint[:, 2 * a:2 * a + 2, tt * P:(tt + 1) * P],
                            rhs=wk[:, 2 * a:2 * a + 2, :], start=(a == 0),
                            stop=(a == 1 and posk is None), perf_mode=DR)
                    if posk:
                        nc.tensor.matmul(kp, lhsT=id_sw, rhs=pkt[:, tt, :],
                                         start=False, stop=True)
                    ek = a16.tile([P, FD], bf16, name=tag + "_ek", tag="kv16", bufs=5)
                    nc.scalar.activation(out=ek, in_=kp, func=AF.Exp, scale=1.0 / SW)
                    vp = pmm.tile([P, FD], f32, name=tag + "_vp", tag="mm")
                    for a in range(2):
                        nc.tensor.matmul(
                            vp, lhsT=vint[:, 2 * a:2 * a + 2, tt * P:(tt + 1) * P],
                            rhs=wv[:, 2 * a:2 * a + 2, :], start=(a == 0),
                            stop=(a == 1), perf_mode=DR)
                    vt = a16.tile([P, H, 132], bf16, name=tag + "_vt", tag="vt16", bufs=5)
                    nc.vector.tensor_scalar_mul(
                        out=vt[:, :, 0:128],
                        in0=vp.rearrange("p (h v) -> p h v", h=H), scalar1=1.0 / SW)
                    nc.vector.memset(vt[:, :, 128:129], 1.0)
                    for h in range(H):
                        nc.tensor.matmul(ctx_ps[h // 2][:, h % 2, 0:129],
                                         lhsT=ek[:, h * P:(h + 1) * P],
                                         rhs=vt[:, h, 0:129],
                                         start=(t == 0 and h % 2 == 0),
                                         stop=(t == 4 * NCH - 1 and h % 2 == 1))
            # ---- phase 2: normalize ctx rows by the ones-column sum ----
            ctx_bf = []
            for h in range(H):
                rk = a32.tile([P, 1], f32, name=tag + "_rk", tag="rk", bufs=4)
                nc.vector.reciprocal(out=rk, in_=ctx_ps[h // 2][:, h % 2, 128:129])
                cb = a16.tile([P, P], bf16, name=tag + "_cbf", tag="cbf", bufs=8)
                nc.vector.tensor_scalar_mul(out=cb, in0=ctx_ps[h // 2][:, h % 2, 0:128],
                                            scalar1=rk[:, 0:1])
                ctx_bf.append(cb)
            # ---- phase 3: qp / q-softmax / att / reproj+resid, chunk ordered ----
            pqv = ct_view(dr[posq]) if posq else None
            rv = dr[resid]
            lst = LNState(tag, zout, final_f32)
            for c in range(NCH):
                qint = load_f8_chunk(dr[qin], c, tag + "_qin")
                pqt = None
                if posq:
                    pqt = a16.tile([P, CT, FD], bf16, name=tag + "_pq", tag="pk16", bufs=2)
                    nc.sync.dma_start(out=pqt, in_=pqv[:, :, c * FD:(c + 1) * FD])
                sq_ps = pcx.tile([P, FD], f32, name=tag + "_sq", tag="cx")
                eq = []
                for m in range(CT):
                    ps = pmm.tile([P, FD], f32, name=tag + "_qp", tag="mm")
                    for a in range(2):
                        nc.tensor.matmul(
                            ps, lhsT=wq[:, 2 * a:2 * a + 2, m * P:(m + 1) * P],
                            rhs=qint[:, 2 * a:2 * a + 2, :], start=(a == 0),
                            stop=(a == 1 and posq is None), perf_mode=DR)
                    if posq:
                        nc.tensor.matmul(ps, lhsT=id_sw, rhs=pqt[:, m, :],
                                         start=False, stop=True)
                    e = a16.tile([P, FD], bf16, name=tag + "_eq", tag="eq16", bufs=6)
                    nc.scalar.activation(out=e, in_=ps, func=AF.Exp, scale=1.0 / SW)
                    eq.append(e)
                    nc.tensor.matmul(sq_ps[32 * m:32 * m + 32, :], lhsT=ones32,
                                     rhs=e, start=True, stop=True,
                                     tile_position=(0, 32 * m))
                # 1/sq = exp(-ln(sq)) on ScalarE over the whole slab tile
                # (every partition holds a valid copy of its head's sums),
                # then per-head gpsimd partition_broadcast. No DRAM trip.
                rql = a32.tile([P, FD], f32, name=tag + "_rql", tag="rqs", bufs=2)
                nc.scalar.activation(out=rql, in_=sq_ps, func=AF.Ln)
                rqe = a16.tile([P, FD], bf16, name=tag + "_rqe", tag="rq16b", bufs=2)
                nc.scalar.activation(out=rqe, in_=rql, func=AF.Exp, scale=-1.0,
                                     bias=lnsa_t[:, 0:1])
                rqb = []
                for h in range(H):
                    # HW partition_broadcast reads absolute partition 0 only;
                    # hop rows at partition 32h down via a 1KB SBUF-SBUF DMA.
                    if h == 0:
                        src = rqe[0:1, :]
                    else:
                        r0 = a16.tile([1, FD], bf16, name=tag + "_rq0_%d" % h,
                                      tag="rq0", bufs=6)
                        nc.scalar.dma_start(out=r0, in_=rqe[32 * h:32 * h + 1, :])
                        src = r0
                    b_ = a16.tile([P, FD], bf16, name=tag + "_rqb%d" % h,
                                  tag="bcr2", bufs=5)
                    nc.gpsimd.partition_broadcast(b_, src)
                    rqb.append(b_)
                if f8r:
                    att = []
                    for hp in range(2):
                        abp = a16.tile([P, 2, FD], f8, name=tag + "_abp%d" % hp,
                                       tag="att8", bufs=4)
                        att.append(abp)
                    for h in range(H):
                        aps = pmm.tile([P, FD], f32, name=tag + "_aps", tag="mm")
                        nc.tensor.matmul(aps, lhsT=ctx_bf[h], rhs=eq[h],
                                         start=True, stop=True)
                        nc.vector.tensor_mul(out=att[h // 2][:, h % 2, :],
                                             in0=aps, in1=rqb[h])
                else:
                    att = []
                    for h in range(H):
                        aps = pmm.tile([P, FD], f32, name=tag + "_aps", tag="mm")
                        nc.tensor.matmul(aps, lhsT=ctx_bf[h], rhs=eq[h],
                                         start=True, stop=True)
                        ab = a16.tile([P, FD], bf16, name=tag + "_ab",
                                      tag="att16", bufs=6)
                        nc.vector.tensor_mul(out=ab, in0=aps, in1=rqb[h])
                        att.append(ab)
                rts = load_ct_chunk(rv, c, tag + "_rt")
                hbc, hsqc = [], []
                for i in range(CT):
                    ps = pmm.tile([P, FD], f32, name=tag + "_rp", tag="mm")
                    if f8r:
                        for a in range(2):
                            nc.tensor.matmul(
                                ps, lhsT=wr[:, 2 * a:2 * a + 2, i * P:(i + 1) * P],
                                rhs=att[a], start=(a == 0), stop=False,
                                perf_mode=DR)
                        nc.tensor.matmul(ps, lhsT=id_swa, rhs=rts[:, i, :],
                                         start=False, stop=True)
                        sc = 1.0 / (SW * SA)
                    else:
                        for hh in range(CT):
                            nc.tensor.matmul(ps, lhsT=wr_b[:, hh, i * P:(i + 1) * P],
                                             rhs=att[hh], start=(hh == 0),
                                             stop=False)
                        nc.tensor.matmul(ps, lhsT=id_sa, rhs=rts[:, i, :],
                                         start=False, stop=True)
                        sc = 1.0 / SA
                    hb_, sq_ = evict_h(tag, ps, sc)
                    hbc.append(hb_)
                    hsqc.append(sq_)
                ln_chunk(lst, hbc, hsqc, c)

        # ---------------- MLP ----------------
        def mlp(tag, zin, zout, final_f32, b1=None, b2=None, f8m=True):
            lst = LNState(tag, zout, final_f32)
            for c in range(NCH):
                zint = load_ct_chunk(dr[zin], c, tag + "_zin")
                rts = zint  # resid source == fc1 input: reuse the same tile
                f2 = [pmm.tile([P, FD], f32, name=tag + "_f2_%d" % i, tag="mm")
                      for i in range(CT)]
                if f8m:
                    zint8 = a16.tile([P, CT, FD], f8, name=tag + "_z8",
                                     tag="ld8", bufs=4)
                    nc.vector.tensor_copy(out=zint8, in_=zint)
                    up = None
                    for ht in range(HT):
                        ps = pcx.tile([P, FD], f32, name=tag + "_f1", tag="cx")
                        for a in range(2):
                            nc.tensor.matmul(
                                ps,
                                lhsT=w1sb[:, 2 * a:2 * a + 2, ht * P:(ht + 1) * P],
                                rhs=zint8[:, 2 * a:2 * a + 2, :], start=(a == 0),
                                stop=(a == 1), perf_mode=DR)
                        if ht % 2 == 0:
                            up = a16.tile([P, 2, FD], f8, name=tag + "_u",
                                          tag="u8", bufs=4)
                        if ht % 4 < 2:
                            nc.scalar.activation(out=up[:, ht % 2, :], in_=ps,
                                                 func=AF.Relu, scale=1.0 / SW)
                        else:
                            nc.vector.tensor_scalar(out=up[:, ht % 2, :], in0=ps,
                                                    scalar1=0.0, scalar2=1.0 / SW,
                                                    op0=ALU.max, op1=ALU.mult)
                        if ht % 2 == 1:
                            for i in range(CT):
                                nc.tensor.matmul(
                                    f2[i],
                                    lhsT=w2sb[:, ht - 1:ht + 1, i * P:(i + 1) * P],
                                    rhs=up, start=(ht == 1), stop=False,
                                    perf_mode=DR)
                    rsc = 1.0 / SW
                    rid = id_sw
                else:
                    for ht in range(HT):
                        ps = pcx.tile([P, FD], f32, name=tag + "_f1", tag="cx")
                        for i in range(CT):
                            nc.tensor.matmul(ps,
                                             lhsT=w1sb_b[:, i, ht * P:(ht + 1) * P],
                                             rhs=zint[:, i, :], start=(i == 0),
                                             stop=(i == CT - 1))
                        ut = a16.tile([P, FD], bf16, name=tag + "_u", tag="u16",
                                      bufs=6)
                        if ht % 2 == 0:
                            nc.scalar.activation(out=ut, in_=ps, func=AF.Relu)
                        else:
                            nc.vector.tensor_scalar_max(out=ut, in0=ps, scalar1=0.0)
                        for i in range(CT):
                            nc.tensor.matmul(f2[i],
                                             lhsT=w2sb_b[:, ht, i * P:(i + 1) * P],
                                             rhs=ut, start=(ht == 0), stop=False)
                    rsc = 1.0
                    rid = id_bf
                hbc, hsqc = [], []
                for i in range(CT):
                    nc.tensor.matmul(f2[i], lhsT=rid, rhs=rts[:, i, :],
                                     start=False, stop=True)
                    hb_, sq_ = evict_h(tag, f2[i], rsc)
                    hbc.append(hb_)
                    hsqc.append(sq_)
                ln_chunk(lst, hbc, hsqc, c)

        bb = lambda n: (n if n in biases else None)

        eattn("xsa", "xP_f8", "xP_f8", "xT_f8", "sa", None, None,
              "xT_bf", dr["z_osa"],
              bq=bb("sa_q"), bk=bb("sa_k"), bv=bb("sa_v"), br=bb("sa_r"))
        eattn("ysa", "yP_f8", "yP_f8", "yT_f8", "sa", None, None,
              "yT_bf", dr["z_ysa"],
              bq=bb("sa_q"), bk=bb("sa_k"), bv=bb("sa_v"), br=bb("sa_r"))
        eattn("xca", "qP_f8", "z_osa", "z_osa", "ca", None, "pk_ca_x",
              "z_osa", dr["z_oca"],
              bq=bb("ca_q"), bk=bb("ca_k"), bv=bb("ca_v"), br=bb("ca_r"))
        mlp("xml", "z_oca", dr["z_oo"], False, b1=bb("mlp1"), b2=bb("mlp2"))
        yca_f8r = os.environ.get("K_YCA_BF16R") != "1"
        yml_f8 = os.environ.get("K_YML_BF16") != "1"
        eattn("yca", "z_oo", "z_ysa", "z_ysa", "ca", "pq_ca_y", "pk_ca_y",
              "z_ysa", dr["z_yca"],
              bq=bb("ca_q"), bk=bb("ca_k"), bv=bb("ca_v"), br=bb("ca_r"),
              f8r=yca_f8r, wr_b=wr_ca_b)
        mlp("yml", "z_yca", out_d, True, b1=bb("mlp1"), b2=bb("mlp2"),
            f8m=yml_f8)

    nc.compile()
    return nc


# ======================= host side =======================

_NC_CACHE = {}
LAST_RESULT = None


def _get_nc(N, ln_affine, biases):
    key = (N, ln_affine, tuple(sorted(biases)))
    if key not in _NC_CACHE:
        _NC_CACHE[key] = build_nc(N, ln_affine, frozenset(biases))
    return _NC_CACHE[key]


def _bf(a):
    return np.ascontiguousarray(a.astype(ml_dtypes.bfloat16))


def _f8(a):
    return np.ascontiguousarray(a.astype(ml_dtypes.float8_e4m3))


def host_prep(inputs, N):
    """Common (core-independent) in_map entries."""
    ws = {w: np.asarray(inputs[w + "_w"], np.float32) for w in ATTN_W}
    posx = np.asarray(inputs["pos_x"], np.float32)[0]  # (N, C)
    posy = np.asarray(inputs["pos_y"], np.float32)[0]
    m = {}
    for w in ATTN_W:
        m[w + "_w8"] = _f8(ws[w] * 32.0)
    m["mlp_w1_8"] = _f8(np.asarray(inputs["mlp_w1"], np.float32) * 32.0)
    m["mlp_w2_8"] = _f8(np.asarray(inputs["mlp_w2"], np.float32) * 32.0)
    m["mlp_w1"] = _bf(np.asarray(inputs["mlp_w1"], np.float32))
    m["mlp_w2"] = _bf(np.asarray(inputs["mlp_w2"], np.float32))
    m["ca_r_wb"] = _bf(ws["ca_r"])
    m["pq_ca_y"] = _bf((posy @ ws["ca_q"]).T)
    m["pk_ca_x"] = _bf(posx @ ws["ca_k"])
    m["pk_ca_y"] = _bf(posy @ ws["ca_k"])
    bias_arr = {"sa_q": "sa_q_b", "sa_k": "sa_k_b", "sa_v": "sa_v_b",
                "sa_r": "sa_r_b", "ca_q": "ca_q_b", "ca_k": "ca_k_b",
                "ca_v": "ca_v_b", "ca_r": "ca_r_b",
                "mlp1": "mlp_b1", "mlp2": "mlp_b2"}
    biases = set()
    for bn, an in bias_arr.items():
        arr = np.asarray(inputs[an], np.float32)
        if np.any(arr != 0):
            biases.add(bn)
            m["b_" + bn] = _bf(arr.reshape(1, -1))
    g = np.asarray(inputs["ln_g"], np.float32)
    b = np.asarray(inputs["ln_b"], np.float32)
    ln_affine = bool(np.any(g != 1) or np.any(b != 0))
    if ln_affine:
        m["ln_g"] = np.ascontiguousarray(g)
        m["ln_b"] = np.ascontiguousarray(b)
    return m, biases, ln_affine


def core_inputs(inputs, b, posx, posy):
    x = np.asarray(inputs["x"], np.float32)[b]
    y = np.asarray(inputs["y"], np.float32)[b]
    q = np.asarray(inputs["q"], np.float32)[b]
    return {"xT_bf": _bf(x.T), "yT_bf": _bf(y.T),
            "xT_f8": _f8(x.T), "yT_f8": _f8(y.T),
            "xP_f8": _f8((x + posx).T), "yP_f8": _f8((y + posy).T),
            "qP_f8": _f8((q + posx).T)}


def kernel(**inputs):
    from concourse import bass_utils
    N = np.asarray(inputs["x"]).shape[1]
    B = np.asarray(inputs["x"]).shape[0]
    common, biases, ln_affine = host_prep(inputs, N)
    nc = _get_nc(N, ln_affine, biases)
    posx = np.asarray(inputs["pos_x"], np.float32)[0]
    posy = np.asarray(inputs["pos_y"], np.float32)[0]
    in_maps = []
    for b in range(B):
        m = dict(common)
        m.update(core_inputs(inputs, b, posx, posy))
        in_maps.append(m)
    res = bass_utils.run_bass_kernel_spmd(nc, in_maps, core_ids=list(range(B)))
    global LAST_RESULT
    LAST_RESULT = res
    out = np.stack([r["yOT"].T for r in res.results], axis=0)
    return np.ascontiguousarray(out.astype(np.float32))
